# revision 16
# baseline (speedup 1.0000x reference)
"""Trainium2 Bass kernel for ConditionDenseCapsule EM routing.

Problem: pose [2,4096,32,16], activation [2,4096,32,1], EM routing with
J=32 output capsules, 3 iterations. Output: capsules [2,32,17] (x2).

Strategy (votes tensor [B,N,J,16] = 512MB is never materialized):
  votes[n,j,pr] = sum_q w[c,j,p,q] u[n,(q,r)]  with n=(t,c)
  All EM-routing quantities are expressed through moments of
  U2U(n) = [1 | u(16) | u2(40)] where u2 = sym pairs u_q*u_r:
    M-step: G/H moments = sum_t r_a[t,j] * U2U[t,f]   (PE matmul per tile)
            then contracted with w / w*w (tiny j-major DVE ops) -> S0/S1/S2m
            -> 8.4KB AllReduce over 8 cores -> mu/sigma2/a_j
    E-step: logits[n,j] = sum_f U2U[f,n] * WALL[c,f,j] (PE matmul per tile)
            WALL = per-iteration coefficient tensor built from mu/sigma2/w.
  softmax over j needs no max-subtraction (logit max verified in [6,35])
  fp16 matmul operands / fp32 PSUM+stats: verified 8.2e-4 rel err in numpy.

Sharding: channel axis CH=32 -> 4 channels per core x 8 cores.
Per-core tiles: (b, c, tdiv) = 2*4*32 tiles of 128 capsules.
"""

import os
import sys
import math
import numpy as np

for _p in ("/root/.axon_site/_ro/trn_rl_repo", "/opt/trn_rl_repo"):
    if _p not in sys.path and os.path.isdir(_p):
        sys.path.append(_p)

import concourse.bass as bass
import concourse.bacc as bacc
import concourse.mybir as mybir
import concourse.tile as tile
from concourse.bass_utils import run_bass_kernel_spmd

F32 = mybir.dt.float32
F16 = mybir.dt.float16
AF = mybir.ActivationFunctionType
ALU = mybir.AluOpType
AX = mybir.AxisListType

B, NI, CH, J, D = 2, 4096, 32, 32, 4
D2 = D * D
N_ITER = 3
EPS = 1e-6
N_CORES = 8
CPC = CH // N_CORES          # channels per core = 4
P = 128                      # partitions / tile size along t
C_SHIFT = 33.0               # constant softmax-logit shift (C[j] ~ 33)

PAIRS = [(q, qp) for q in range(D) for qp in range(q, D)]   # 10, group-major
NPAIR = len(PAIRS)
PAIR_W = np.array([1.0 if q == qp else 2.0 for (q, qp) in PAIRS], np.float32)
# ww2 rows grouped by first index q: group q has pairs (q,q'>=q)
Q_GROUP = [[k for k, (q, qp) in enumerate(PAIRS) if q == g] for g in range(D)]

# feature layout (rows of U2U / cols of V / rows of WALL), 97 wide:
#   [0:40)  u2 sym-pair products    [40:64) zero pad
#   [64:80) u                       [80:96) zero pad
#   96      ones (C row in WALL)
# pads keep every SBUF row-slice anchored at partition 0/32/64/96.
F_U2, F_U, F_ONE, VF = 0, 64, 96, 97
NST = 33                     # reduced stats per b: S0 | S1(16) | S2m(16)


def _build_nc(ni=NI, cpc=CPC, n_cores=N_CORES, collective=True):
    """Build the Bass module (SPMD, same NEFF on every core)."""
    tdiv = ni // P               # t-blocks of 128
    ntile = B * cpc * tdiv       # tiles per core
    ngrp = tdiv // 4             # logits groups of 4 tiles per (b,c)
    assert tdiv % 4 == 0

    nc = bacc.Bacc("TRN2", target_bir_lowering=False)

    pose_s = nc.declare_dram_parameter("pose_s", [B, ni, cpc, D2], F32, isOutput=False)
    act_s = nc.declare_dram_parameter("act_s", [B, ni, cpc], F32, isOutput=False)
    w_jm = nc.declare_dram_parameter("w_jm", [J, D * cpc * D], F32, isOutput=False)
    ww_jm = nc.declare_dram_parameter("ww_jm", [J, D * cpc * NPAIR], F32, isOutput=False)
    w_f = nc.declare_dram_parameter("w_f", [D2, D * cpc * J], F32, isOutput=False)
    ww2_f = nc.declare_dram_parameter("ww2_f", [4 * NPAIR, D * cpc * J], F32, isOutput=False)
    beta_v_j = nc.declare_dram_parameter("beta_v_j", [J, D2], F32, isOutput=False)
    beta_a_j = nc.declare_dram_parameter("beta_a_j", [J, 1], F32, isOutput=False)
    id128h = nc.declare_dram_parameter("id128h", [P, P], F16, isOutput=False)
    qsel = nc.declare_dram_parameter("qsel", [J, B * D * D2], F16, isOutput=False)
    sel40 = nc.declare_dram_parameter("sel40", [J, B * D * 4 * NPAIR], F16, isOutput=False)
    ones32 = nc.declare_dram_parameter("ones32", [1, J], F16, isOutput=False)
    caps = nc.declare_dram_parameter("caps", [B, J, D2 + 1], F32, isOutput=True)

    lam = [0.01 * (1.0 - 0.95 ** (t + 1)) for t in range(N_ITER)]

    with tile.TileContext(nc) as tc:
        with (
            tc.tile_pool(name="persist", bufs=1) as pp,
            tc.tile_pool(name="work", bufs=3) as wp,
            tc.tile_pool(name="wall", bufs=2) as wallp,
            tc.tile_pool(name="ps_tr", bufs=2, space="PSUM") as ps_tr,
            tc.tile_pool(name="ps_lg", bufs=2, space="PSUM") as ps_lg,
            tc.tile_pool(name="ps_st", bufs=1, space="PSUM") as ps_st,
            tc.tile_pool(name="ps_misc", bufs=1, space="PSUM") as ps_misc,
            tc.tile_pool(name="dram", bufs=2, space="DRAM") as dramp,
        ):
            # ---------- persistent SBUF ----------
            V = pp.tile([P, ntile * VF], F16)          # n-major features
            U2U = pp.tile([VF, ntile * P], F16)        # f-major transpose
            u_stage = pp.tile([P, ntile * D2], F32)
            acts = pp.tile([P, B * cpc * tdiv], F32)
            actsJ = pp.tile([P, B * cpc * tdiv], F16)
            sb_wjm = pp.tile([J, D * cpc * D], F32)
            sb_wwjm = pp.tile([J, D * cpc * NPAIR], F32)
            sb_wf = pp.tile([D2, D * cpc * J], F32)
            sb_ww2f = pp.tile([4 * NPAIR, D * cpc * J], F32)
            sb_bv = pp.tile([J, D2], F32)
            sb_ba = pp.tile([J, 1], F32)
            sb_id = pp.tile([P, P], F16)
            sb_qsel = pp.tile([J, B * D * D2], F16)
            sb_sel40 = pp.tile([J, B * D * 4 * NPAIR], F16)
            sb_ones32 = pp.tile([1, J], F16)
            bvsum = pp.tile([J, 1], F32)
            stats_sb = pp.tile([J, B * cpc * VF], F32)
            m0_sb = pp.tile([1, B * cpc * VF], F16)
            sred = pp.tile([J, B * NST], F32)
            sall = pp.tile([J, B * NST], F32)
            cscr = pp.tile([J, B * J], F32)
            ct = pp.tile([J, B * J], F32)
            out_sb = pp.tile([J, B * (D2 + 1)], F32)
            # per-iteration small tensors (overwritten each iter)
            s0e = pp.tile([J, B], F32)
            s0r = pp.tile([J, B], F32)
            mu = pp.tile([J, B * D2], F32)
            t_a = pp.tile([J, B * D2], F32)
            t_b = pp.tile([J, B * D2], F32)
            sig = pp.tile([J, B * D2], F32)
            lsig = pp.tile([J, B * D2], F32)
            isig = pp.tile([J, B * D2], F32)
            m1 = pp.tile([J, B * D2], F32)
            m1t = pp.tile([J, B * D2], F32)
            isigt = pp.tile([J, B * D2], F32)
            m1th = pp.tile([J, B * D2], F16)
            isigth = pp.tile([J, B * D2], F16)
            ls = pp.tile([J, B], F32)
            costf = pp.tile([J, B], F32)
            costt = pp.tile([J, B], F32)
            a_j = pp.tile([J, B], F32)
            la = pp.tile([J, B], F32)
            cj = pp.tile([J, B], F32)
            ba_l = pp.tile([J, 1], F32)
            eps1 = pp.tile([J, 1], F32)
            csh1 = pp.tile([J, 1], F32)
            tmp_u = pp.tile([D2, B * cpc * J * D], F32)
            tmp_ur = pp.tile([D2, B * cpc * J], F32)
            tmp_2 = pp.tile([4 * NPAIR, B * cpc * J * D], F32)
            tmp_2r = pp.tile([4 * NPAIR, B * cpc * J], F32)
            tt_c = pp.tile([J, B * D * cpc * D], F32)       # contraction tmp u
            tt_h = pp.tile([J, B * D * cpc * NPAIR], F32)   # contraction tmp u2

            # PSUM persistents: per-b stats tiles (+ reused for M0 row 0)
            stats_ps = [ps_st.tile([J, cpc * VF], F32, name=f"stps{bb}")
                        for bb in range(B)]
            qbf1_ps = ps_misc.tile([D2, B * D * J], F32, name="qbf1")
            qbf2_ps = ps_misc.tile([4 * NPAIR, B * D * J], F32, name="qbf2")

            # ---------- load ----------
            usv = u_stage.rearrange("p (b c td f) -> p b c td f",
                                    b=B, c=cpc, td=tdiv)
            psv = pose_s.rearrange("b (td tm) c f -> tm b c td f", tm=P)
            for bb in range(B):
                for cc in range(cpc):
                    nc.sync.dma_start(out=usv[:, bb, cc], in_=psv[:, bb, cc])
            acv = acts.rearrange("p (b c td) -> p b c td", b=B, c=cpc)
            asv = act_s.rearrange("b (td tm) c -> tm b c td", tm=P)
            for bb in range(B):
                nc.sync.dma_start(out=acv[:, bb], in_=asv[:, bb])
            for dst, src in [(sb_wjm, w_jm), (sb_wwjm, ww_jm), (sb_wf, w_f),
                             (sb_ww2f, ww2_f), (sb_bv, beta_v_j),
                             (sb_ba, beta_a_j), (sb_id, id128h),
                             (sb_qsel, qsel), (sb_sel40, sel40),
                             (sb_ones32, ones32)]:
                nc.sync.dma_start(out=dst[:, :], in_=src[:, :])

            Vv = V.rearrange("p (n f) -> p n f", f=VF)
            # zero pads, ones col, u cols (fp16), u2 products
            nc.vector.memset(Vv[:, :, 40:64], 0.0)
            nc.vector.memset(Vv[:, :, 80:96], 0.0)
            nc.vector.memset(Vv[:, :, F_ONE:F_ONE + 1], 1.0)
            Vv5 = V.rearrange("p (b c td f) -> p b c td f",
                              b=B, c=cpc, td=tdiv)
            for bb in range(B):
                for cc in range(cpc):
                    nc.vector.tensor_copy(
                        Vv5[:, bb, cc, :, F_U:F_U + D2],
                        usv[:, bb, cc])
            for k, (q, qp) in enumerate(PAIRS):
                nc.vector.tensor_mul(
                    Vv[:, :, 4 * k: 4 * k + 4],
                    Vv[:, :, F_U + 4 * q: F_U + 4 * q + 4],
                    Vv[:, :, F_U + 4 * qp: F_U + 4 * qp + 4])
            aJv = actsJ.rearrange("p (b c td) -> p b c td", b=B, c=cpc)
            for bb in range(B):
                nc.vector.tensor_scalar_mul(aJv[:, bb], acv[:, bb], 1.0 / J)
            nc.vector.tensor_reduce(bvsum[:, :], sb_bv[:, :], axis=AX.X,
                                    op=ALU.add)
            nc.vector.memset(cscr[:, :], 0.0)
            nc.vector.memset(eps1[:, :], EPS)
            nc.vector.memset(csh1[:, :], -C_SHIFT)

            # ---------- helpers ----------
            def contraction(src_sb):
                """stats [J,(b,c,VF)] j-major -> sred [J,(b,33)] = S0|S1|S2m"""
                sv = src_sb.rearrange("p (b c f) -> p b c f", b=B, c=cpc)
                rv = sred.rearrange("p (b f) -> p b f", b=B)
                w4 = sb_wjm.rearrange("p (pp c q) -> p pp c q", pp=D, c=cpc)
                ww4 = sb_wwjm.rearrange("p (pp c k) -> p pp c k", pp=D, c=cpc)
                tcv = tt_c.rearrange("p (b r c q) -> p b r c q",
                                     b=B, r=D, c=cpc)
                thv = tt_h.rearrange("p (b r c k) -> p b r c k",
                                     b=B, r=D, c=cpc)
                nc.vector.tensor_reduce(rv[:, :, 0:1],
                                        sv[:, :, :, F_ONE:F_ONE + 1],
                                        axis=AX.XY, op=ALU.add)
                for bb in range(B):
                    for pidx in range(D):
                        # S1[b,j,p,r] = sum_{c,q} w[c,j,p,q] G[b,c,j,(q r)]
                        in0 = sv[:, bb, :, F_U:F_U + D2] \
                            .rearrange("p c (q r) -> p r c q", q=D)
                        in1 = w4[:, pidx].unsqueeze(1) \
                            .broadcast_to((J, D, cpc, D))
                        t0 = tcv[:, bb]
                        nc.vector.tensor_tensor(t0, in0, in1, op=ALU.mult)
                        nc.vector.tensor_reduce(
                            rv[:, bb, 1 + 4 * pidx:1 + 4 * pidx + 4], t0,
                            axis=AX.XY, op=ALU.add)
                        # S2m[b,j,p,r] = sum_{c,k} ww[c,j,p,k] H[b,c,j,(k r)]
                        in0h = sv[:, bb, :, F_U2:F_U2 + 4 * NPAIR] \
                            .rearrange("p c (k r) -> p r c k", k=NPAIR)
                        in1h = ww4[:, pidx].unsqueeze(1) \
                            .broadcast_to((J, D, cpc, NPAIR))
                        t1 = thv[:, bb]
                        nc.vector.tensor_tensor(t1, in0h, in1h, op=ALU.mult)
                        nc.vector.tensor_reduce(
                            rv[:, bb, 17 + 4 * pidx:17 + 4 * pidx + 4], t1,
                            axis=AX.XY, op=ALU.add)

            def allreduce(it):
                din = dramp.tile([J, B * NST], F32, name=f"din{it}")
                dout = dramp.tile([J, B * NST], F32, name=f"dout{it}")
                nc.sync.dma_start(out=din[:, :], in_=sred[:, :])
                if collective:
                    nc.gpsimd.collective_compute(
                        "AllReduce", ALU.add,
                        replica_groups=[list(range(n_cores))],
                        ins=[din.opt()], outs=[dout.opt()])
                    nc.sync.dma_start(out=sall[:, :], in_=dout[:, :])
                else:
                    nc.sync.dma_start(out=sall[:, :], in_=din[:, :])

            def post_stats(t, build_wall):
                sv = sall.rearrange("p (b f) -> p b f", b=B)
                s0v = sv[:, :, 0]
                s1v = sv[:, :, 1:17]
                s2v = sv[:, :, 17:33]
                v3 = lambda x: x.rearrange("p (b f) -> p b f", b=B)
                b16 = lambda x: x.unsqueeze(2).broadcast_to((J, B, D2))
                nc.vector.tensor_scalar_add(s0e[:, :], s0v, EPS)
                nc.vector.reciprocal(s0r[:, :], s0e[:, :])
                nc.vector.tensor_tensor(v3(mu), s1v, b16(s0r), op=ALU.mult)
                # sigma2 = (S2m - mu*(2*S1 - mu*S0)) / S0e + EPS
                nc.vector.tensor_tensor(v3(t_a), v3(mu), b16(s0v),
                                        op=ALU.mult)
                nc.vector.tensor_scalar_mul(v3(t_b), s1v, 2.0)
                nc.vector.tensor_sub(t_b[:, :], t_b[:, :], t_a[:, :])
                nc.vector.tensor_tensor(t_a[:, :], mu[:, :], t_b[:, :],
                                        op=ALU.mult)
                nc.vector.tensor_sub(v3(t_a), s2v, v3(t_a))
                nc.vector.tensor_tensor(v3(sig), v3(t_a), b16(s0r),
                                        op=ALU.mult)
                nc.vector.tensor_scalar_add(sig[:, :], sig[:, :], EPS)
                nc.scalar.activation(lsig[:, :], sig[:, :], AF.Ln)
                nc.vector.reciprocal(isig[:, :], sig[:, :])
                nc.vector.tensor_reduce(
                    ls[:, :], lsig.rearrange("p (b f) -> p b f", b=B),
                    axis=AX.X, op=ALU.add)
                # cost_total = S0 * (0.5*sum(log sig) + sum(beta_v))
                nc.scalar.activation(costf[:, :], ls[:, :], AF.Identity,
                                     bias=bvsum[:, 0:1], scale=0.5)
                nc.vector.tensor_tensor(costt[:, :], costf[:, :], s0v,
                                        op=ALU.mult)
                nc.scalar.mul(ba_l[:, :], sb_ba[:, :], float(lam[t]))
                nc.scalar.activation(a_j[:, :], costt[:, :], AF.Sigmoid,
                                     bias=ba_l[:, 0:1], scale=-float(lam[t]))
                if not build_wall:
                    return None
                wall = wallp.tile([VF, B * cpc * J], F16, name="wall")
                nc.vector.memset(wall[:, :], 0.0)
                nc.vector.tensor_mul(m1[:, :], mu[:, :], isig[:, :])
                nc.scalar.activation(la[:, :], a_j[:, :], AF.Ln,
                                     bias=eps1[:, 0:1])
                # C = log(a+eps) - 0.5*sum(mu*m1 + lsig) - C_SHIFT
                nc.vector.tensor_mul(t_a[:, :], mu[:, :], m1[:, :])
                nc.vector.tensor_add(t_a[:, :], t_a[:, :], lsig[:, :])
                nc.vector.tensor_reduce(
                    cj[:, :], t_a.rearrange("p (b f) -> p b f", b=B),
                    axis=AX.X, op=ALU.add)
                nc.scalar.activation(cj[:, :], cj[:, :], AF.Identity,
                                     bias=csh1[:, 0:1], scale=-0.5)
                nc.vector.tensor_add(cj[:, :], cj[:, :], la[:, :])
                # C[b] to row 0 of per-b 32-blocks via stream transpose
                cv = cscr.rearrange("p (b j) -> p b j", b=B)
                for bb in range(B):
                    nc.vector.tensor_copy(cv[:, bb, 0:1],
                                          cj[:, bb:bb + 1])
                nc.vector.transpose(ct[:, :], cscr[:, :])
                wv = wall.rearrange("p (b c j) -> p b c j", b=B, c=cpc)
                ctv = ct.rearrange("p (b j) -> p b j", b=B)
                for bb in range(B):
                    nc.vector.tensor_copy(
                        wv[F_ONE:F_ONE + 1, bb],
                        ctv[0:1, bb].unsqueeze(1)
                        .broadcast_to((1, cpc, J)))
                # M1 / ISIG transposed [32 (b,p,r), 32 j]
                nc.vector.transpose(m1t[:, :], m1[:, :])
                nc.vector.transpose(isigt[:, :], isig[:, :])
                nc.vector.tensor_copy(m1th[:, :], m1t[:, :])
                nc.vector.tensor_copy(isigth[:, :], isigt[:, :])
                # broadcast rows via selector matmuls
                q1 = qbf1_ps.rearrange("p (b pp j) -> p b pp j", b=B, pp=D)
                q2 = qbf2_ps.rearrange("p (b pp j) -> p b pp j", b=B, pp=D)
                for bb in range(B):
                    for pidx in range(D):
                        i0 = (bb * D + pidx) * D2
                        nc.tensor.matmul(
                            q1[:, bb, pidx],
                            sb_qsel[:, i0:i0 + D2],
                            m1th[:, 0:J], start=True, stop=True)
                        i1 = (bb * D + pidx) * 4 * NPAIR
                        nc.tensor.matmul(
                            q2[:, bb, pidx],
                            sb_sel40[:, i1:i1 + 4 * NPAIR],
                            isigth[:, 0:J], start=True, stop=True)
                # WALL u-part: rows 64..80 = sum_p w_f * M1_qbf
                w_f_ap = sb_wf.rearrange("p (pp c j) -> p pp c j",
                                         pp=D, c=cpc).transpose([0, 2, 3, 1])
                ww2_ap = sb_ww2f.rearrange("p (pp c j) -> p pp c j",
                                           pp=D, c=cpc).transpose([0, 2, 3, 1])
                tuv = tmp_u.rearrange("p (b c j pp) -> p b c j pp",
                                      b=B, c=cpc, j=J)
                turv = tmp_ur.rearrange("p (b c j) -> p b c j", b=B, c=cpc)
                t2v = tmp_2.rearrange("p (b c j pp) -> p b c j pp",
                                      b=B, c=cpc, j=J)
                t2rv = tmp_2r.rearrange("p (b c j) -> p b c j", b=B, c=cpc)
                for bb in range(B):
                    m1q = q1[:, bb].transpose([0, 2, 1]).unsqueeze(1) \
                        .broadcast_to((D2, cpc, J, D))
                    nc.vector.tensor_tensor(tuv[:, bb], w_f_ap, m1q,
                                            op=ALU.mult)
                    nc.vector.tensor_reduce(turv[:, bb], tuv[:, bb],
                                            axis=AX.X, op=ALU.add)
                    # WALL u2-part: rows 0..40 = sum_p ww2_f * ISIG_pair
                    isq = q2[:, bb].transpose([0, 2, 1]).unsqueeze(1) \
                        .broadcast_to((4 * NPAIR, cpc, J, D))
                    nc.vector.tensor_tensor(t2v[:, bb], ww2_ap, isq,
                                            op=ALU.mult)
                    nc.vector.tensor_reduce(t2rv[:, bb], t2v[:, bb],
                                            axis=AX.X, op=ALU.add)
                nc.vector.tensor_copy(wall[F_U:F_U + D2, :], tmp_ur[:, :])
                nc.vector.tensor_copy(wall[F_U2:F_U2 + 4 * NPAIR, :],
                                      tmp_2r[:, :])
                return wall

            # ---------- phase 0: M-step with uniform r ----------
            for bb in range(B):
                for cc in range(cpc):
                    for td in range(tdiv):
                        i = (bb * cpc + cc) * tdiv + td
                        nc.tensor.matmul(
                            stats_ps[bb][0:1, cc * VF:(cc + 1) * VF],
                            actsJ[:, i:i + 1], V[:, i * VF:(i + 1) * VF],
                            start=(td == 0), stop=(td == tdiv - 1))
            m0v = m0_sb.rearrange("p (b f) -> p b f", b=B)
            for bb in range(B):
                nc.vector.tensor_copy(m0v[:, bb], stats_ps[bb][0:1, :])
            for bb in range(B):
                nc.tensor.matmul(stats_ps[bb][:, :], sb_ones32[:, :],
                                 m0v[:, bb], start=True, stop=True)
            ssv = stats_sb.rearrange("p (b x) -> p b x", b=B)
            for bb in range(B):
                nc.vector.tensor_copy(ssv[:, bb], stats_ps[bb][:, :])

            # ---------- f-major transposes (overlap with AR0/post0) ----------
            for i in range(ntile):
                tr = ps_tr.tile([VF, P], F16, name="tr")
                nc.tensor.transpose(tr[:, :], V[:, i * VF:(i + 1) * VF],
                                    sb_id[:, :])
                nc.vector.tensor_copy(U2U[:, i * P:(i + 1) * P], tr[:, :])

            contraction(stats_sb)
            allreduce(0)
            wall = post_stats(0, build_wall=True)

            # ---------- EM passes ----------
            av = acts.rearrange("p (b c td) -> p b c td", b=B, c=cpc)
            for t in range(1, N_ITER):
                for bb in range(B):
                    for cc in range(cpc):
                        bc = bb * cpc + cc
                        for g in range(ngrp):
                            lg = ps_lg.tile([P, 4 * J], F32, name="lg")
                            for k in range(4):
                                i = bc * tdiv + g * 4 + k
                                nc.tensor.matmul(
                                    lg[:, k * J:(k + 1) * J],
                                    U2U[:, i * P:(i + 1) * P],
                                    wall[:, bc * J:(bc + 1) * J],
                                    start=True, stop=True)
                            e = wp.tile([P, 4 * J], F32, name="e")
                            nc.scalar.activation(e[:, :], lg[:, :], AF.Exp)
                            ev = e.rearrange("p (g j) -> p g j", g=4)
                            rs = wp.tile([P, 4], F32, name="rs")
                            nc.vector.tensor_reduce(rs[:, :], ev, axis=AX.X,
                                                    op=ALU.add)
                            rsi = wp.tile([P, 4], F32, name="rsi")
                            nc.vector.reciprocal(rsi[:, :], rs[:, :])
                            sc = wp.tile([P, 4], F32, name="sc")
                            nc.vector.tensor_mul(
                                sc[:, :], rsi[:, :],
                                av[:, bb, cc, g * 4:g * 4 + 4])
                            ra = wp.tile([P, 4 * J], F16, name="ra")
                            nc.vector.tensor_tensor(
                                ra.rearrange("p (g j) -> p g j", g=4), ev,
                                sc.unsqueeze(2).broadcast_to((P, 4, J)),
                                op=ALU.mult)
                            for k in range(4):
                                td = g * 4 + k
                                i = bc * tdiv + td
                                nc.tensor.matmul(
                                    stats_ps[bb][:, cc * VF:(cc + 1) * VF],
                                    ra[:, k * J:(k + 1) * J],
                                    V[:, i * VF:(i + 1) * VF],
                                    start=(td == 0), stop=(td == tdiv - 1))
                for bb in range(B):
                    nc.vector.tensor_copy(ssv[:, bb], stats_ps[bb][:, :])
                contraction(stats_sb)
                allreduce(t)
                wall = post_stats(t, build_wall=(t < N_ITER - 1))

            # ---------- output ----------
            ov = out_sb.rearrange("p (b f) -> p b f", b=B)
            nc.vector.tensor_copy(
                ov[:, :, 0:D2], mu.rearrange("p (b f) -> p b f", b=B))
            nc.vector.tensor_copy(ov[:, :, D2:D2 + 1],
                                  a_j.unsqueeze(2))
            nc.sync.dma_start(out=caps.transpose([1, 0, 2]), in_=ov)

    nc.finalize()
    return nc


def _host_statics(w, beta_v, beta_a, core, cpc=CPC):
    """Per-core static tensors derived from learned params."""
    wc = np.ascontiguousarray(w[core * cpc:(core + 1) * cpc], np.float32)
    # w_jm [J, (p, c, q)]
    w_jm = np.ascontiguousarray(wc.transpose(1, 2, 0, 3)).reshape(J, -1)
    # ww_jm [J, (p, c, k)] with (2-delta) folded
    ww = np.stack([wc[:, :, :, q] * wc[:, :, :, qp] for (q, qp) in PAIRS],
                  axis=-1) * PAIR_W  # [c, J, p, k]
    ww_jm = np.ascontiguousarray(ww.transpose(1, 2, 0, 3)).reshape(J, -1)
    # w_f [(q,r), (p, c, j)] (r-broadcast of w[c,j,p,q])
    w_f = np.empty((D2, D, cpc, J), np.float32)
    for q in range(D):
        for r in range(D):
            # [p, c, j] = wc[c, j, p, q] -> transpose
            w_f[q * 4 + r] = wc[:, :, :, q].transpose(2, 0, 1)
    w_f = w_f.reshape(D2, -1)
    # ww2_f [(k,r) 40, (p, c, j)] values -0.5*(2-delta)*w_q*w_qp
    ww2 = np.empty((4 * NPAIR, D, cpc, J), np.float32)
    for k, (q, qp) in enumerate(PAIRS):
        val = (-0.5 * PAIR_W[k]) * wc[:, :, :, q] * wc[:, :, :, qp]  # [c,J,p]
        for r in range(D):
            ww2[4 * k + r] = val.transpose(2, 0, 1)
    ww2_f = ww2.reshape(4 * NPAIR, -1)
    qsel = np.zeros((J, B * D * D2), np.float32)
    sel40 = np.zeros((J, B * D * 4 * NPAIR), np.float32)
    for b in range(B):
        for p in range(D):
            for r in range(D):
                row = 16 * b + 4 * p + r
                for q in range(D):
                    qsel[row, (b * D + p) * D2 + q * 4 + r] = 1.0
                for k in range(NPAIR):
                    sel40[row, (b * D + p) * 4 * NPAIR + 4 * k + r] = 1.0
    return {
        "w_jm": w_jm, "ww_jm": ww_jm, "w_f": w_f, "ww2_f": ww2_f,
        "beta_v_j": np.ascontiguousarray(beta_v, np.float32).reshape(J, D2),
        "beta_a_j": np.ascontiguousarray(beta_a, np.float32).reshape(J, 1),
        "id128h": np.eye(P, dtype=np.float16),
        "qsel": qsel.astype(np.float16), "sel40": sel40.astype(np.float16),
        "ones32": np.ones((1, J), np.float16),
    }


_CACHE = {}


def kernel(pose, activation, w, beta_v, beta_a):
    pose = np.ascontiguousarray(np.asarray(pose), dtype=np.float32)
    activation = np.ascontiguousarray(np.asarray(activation), np.float32)
    w = np.asarray(w, dtype=np.float32)
    beta_v = np.asarray(beta_v, np.float32)
    beta_a = np.asarray(beta_a, np.float32)

    if "nc" not in _CACHE:
        _CACHE["nc"] = _build_nc()
    nc = _CACHE["nc"]

    in_maps = []
    for core in range(N_CORES):
        sl = slice(core * CPC, (core + 1) * CPC)
        m = {
            "pose_s": np.ascontiguousarray(pose[:, :, sl, :]),
            "act_s": np.ascontiguousarray(activation[:, :, sl, 0]),
        }
        m.update(_host_statics(w, beta_v, beta_a, core))
        in_maps.append(m)

    trace = bool(int(os.environ.get("KERNEL_TRACE", "0")))
    try:
        res = run_bass_kernel_spmd(nc, in_maps,
                                   core_ids=list(range(N_CORES)),
                                   trace=trace)
    except ModuleNotFoundError:
        res = run_bass_kernel_spmd(nc, in_maps,
                                   core_ids=list(range(N_CORES)),
                                   trace=False)
    if trace and res.exec_time_ns is not None:
        print(f"HW exec time: {res.exec_time_ns} ns")
        _CACHE["exec_time_ns"] = res.exec_time_ns
        _CACHE["results"] = res
    caps = np.asarray(res.results[0]["caps"], np.float32)
    return (caps, caps.copy())


if __name__ == "__main__":
    rng = np.random.default_rng(0)
    inputs = {
        "pose": rng.standard_normal((B, NI, CH, D2), dtype=np.float32),
        "activation": rng.random((B, NI, CH, 1), dtype=np.float32),
        "w": 0.1 * rng.standard_normal((CH, J, D, D), dtype=np.float32),
        "beta_v": 0.1 * rng.standard_normal((J, D2), dtype=np.float32),
        "beta_a": 0.1 * rng.standard_normal((J, 1), dtype=np.float32),
    }
    out = kernel(**inputs)
    print(out[0].shape, out[0][0, 0])


# revision 17
# speedup vs baseline: 982121592.9338x; 982121592.9338x over previous
"""Trainium2 Bass kernel for ConditionDenseCapsule EM routing.

Problem: pose [2,4096,32,16], activation [2,4096,32,1], EM routing with
J=32 output capsules, 3 iterations. Output: capsules [2,32,17] (x2).

Strategy (votes tensor [B,N,J,16] = 512MB is never materialized):
  votes[n,j,pr] = sum_q w[c,j,p,q] u[n,(q,r)]  with n=(t,c)
  All EM-routing quantities are expressed through moments of
  U2U(n) = [1 | u(16) | u2(40)] where u2 = sym pairs u_q*u_r:
    M-step: G/H moments = sum_t r_a[t,j] * U2U[t,f]   (PE matmul per tile)
            then contracted with w / w*w (tiny j-major DVE ops) -> S0/S1/S2m
            -> 8.4KB AllReduce over 8 cores -> mu/sigma2/a_j
    E-step: logits[n,j] = sum_f U2U[f,n] * WALL[c,f,j] (PE matmul per tile)
            WALL = per-iteration coefficient tensor built from mu/sigma2/w.
  softmax over j needs no max-subtraction (logit max verified in [6,35])
  fp16 matmul operands / fp32 PSUM+stats: verified 8.2e-4 rel err in numpy.

Sharding: channel axis CH=32 -> 4 channels per core x 8 cores.
Per-core tiles: (b, c, tdiv) = 2*4*32 tiles of 128 capsules.
"""

import os
import sys
import math
import numpy as np

for _p in ("/root/.axon_site/_ro/trn_rl_repo", "/opt/trn_rl_repo"):
    if _p not in sys.path and os.path.isdir(_p):
        sys.path.append(_p)

import concourse.bass as bass
import concourse.bacc as bacc
import concourse.mybir as mybir
import concourse.tile as tile
from concourse.bass_utils import run_bass_kernel_spmd

F32 = mybir.dt.float32
F16 = mybir.dt.float16
AF = mybir.ActivationFunctionType
ALU = mybir.AluOpType
AX = mybir.AxisListType

B, NI, CH, J, D = 2, 4096, 32, 32, 4
D2 = D * D
N_ITER = 3
EPS = 1e-6
N_CORES = 8
CPC = CH // N_CORES          # channels per core = 4
P = 128                      # partitions / tile size along t
C_SHIFT = 33.0               # constant softmax-logit shift (C[j] ~ 33)

PAIRS = [(q, qp) for q in range(D) for qp in range(q, D)]   # 10, group-major
NPAIR = len(PAIRS)
PAIR_W = np.array([1.0 if q == qp else 2.0 for (q, qp) in PAIRS], np.float32)
# ww2 rows grouped by first index q: group q has pairs (q,q'>=q)
Q_GROUP = [[k for k, (q, qp) in enumerate(PAIRS) if q == g] for g in range(D)]

# feature layout (rows of U2U / cols of V / rows of WALL), 97 wide:
#   [0:40)  u2 sym-pair products    [40:64) zero pad
#   [64:80) u                       [80:96) zero pad
#   96      ones (C row in WALL)
# pads keep every SBUF row-slice anchored at partition 0/32/64/96.
F_U2, F_U, F_ONE, VF = 0, 64, 96, 97
NST = 33                     # reduced stats per b: S0 | S1(16) | S2m(16)


def _build_nc(ni=NI, cpc=CPC, n_cores=N_CORES, collective=True):
    """Build the Bass module (SPMD, same NEFF on every core)."""
    tdiv = ni // P               # t-blocks of 128
    ntile = B * cpc * tdiv       # tiles per core
    GW = 8 if tdiv % 8 == 0 else 4   # tiles per logits group
    ngrp = tdiv // GW
    assert tdiv % GW == 0

    nc = bacc.Bacc("TRN2", target_bir_lowering=False)

    pose_s = nc.declare_dram_parameter("pose_s", [B, ni, cpc, D2], F32, isOutput=False)
    act_s = nc.declare_dram_parameter("act_s", [B, ni, cpc], F32, isOutput=False)
    w_jm = nc.declare_dram_parameter("w_jm", [J, D * cpc * D], F32, isOutput=False)
    ww_jm = nc.declare_dram_parameter("ww_jm", [J, D * cpc * NPAIR], F32, isOutput=False)
    w_f = nc.declare_dram_parameter("w_f", [D2, D * cpc * J], F32, isOutput=False)
    ww2_f = nc.declare_dram_parameter("ww2_f", [4 * NPAIR, D * cpc * J], F32, isOutput=False)
    beta_v_j = nc.declare_dram_parameter("beta_v_j", [J, D2], F32, isOutput=False)
    beta_a_j = nc.declare_dram_parameter("beta_a_j", [J, 1], F32, isOutput=False)
    id128h = nc.declare_dram_parameter("id128h", [P, P], F16, isOutput=False)
    qsel = nc.declare_dram_parameter("qsel", [J, B * D * D2], F16, isOutput=False)
    sel40 = nc.declare_dram_parameter("sel40", [J, B * D * 4 * NPAIR], F16, isOutput=False)
    ones32 = nc.declare_dram_parameter("ones32", [1, J], F16, isOutput=False)
    caps = nc.declare_dram_parameter("caps", [B, J, D2 + 1], F32, isOutput=True)

    lam = [0.01 * (1.0 - 0.95 ** (t + 1)) for t in range(N_ITER)]

    with tile.TileContext(nc) as tc:
        with (
            tc.tile_pool(name="persist", bufs=1) as pp,
            tc.tile_pool(name="work", bufs=3) as wp,
            tc.tile_pool(name="wall", bufs=2) as wallp,
            tc.tile_pool(name="ps_tr", bufs=2, space="PSUM") as ps_tr,
            tc.tile_pool(name="ps_lg", bufs=2, space="PSUM") as ps_lg,
            tc.tile_pool(name="ps_st", bufs=1, space="PSUM") as ps_st,
            tc.tile_pool(name="ps_misc", bufs=1, space="PSUM") as ps_misc,
            tc.tile_pool(name="dram", bufs=2, space="DRAM") as dramp,
        ):
            # ---------- persistent SBUF ----------
            V = pp.tile([P, ntile * VF], F16)          # n-major features
            U2U = pp.tile([VF, ntile * P], F16)        # f-major transpose
            u_stage = pp.tile([P, ntile * D2], F32)
            acts = pp.tile([P, B * cpc * tdiv], F32)
            actsJ = pp.tile([P, B * cpc * tdiv], F16)
            sb_wjm = pp.tile([J, D * cpc * D], F32)
            sb_wwjm = pp.tile([J, D * cpc * NPAIR], F32)
            sb_wf = pp.tile([D2, D * cpc * J], F32)
            sb_ww2f = pp.tile([4 * NPAIR, D * cpc * J], F32)
            sb_bv = pp.tile([J, D2], F32)
            sb_ba = pp.tile([J, 1], F32)
            sb_id = pp.tile([P, P], F16)
            sb_qsel = pp.tile([J, B * D * D2], F16)
            sb_sel40 = pp.tile([J, B * D * 4 * NPAIR], F16)
            sb_ones32 = pp.tile([1, J], F16)
            bvsum = pp.tile([J, 1], F32)
            stats_sb = pp.tile([J, B * cpc * VF], F32)
            m0_sb = pp.tile([1, B * cpc * VF], F16)
            sred = pp.tile([J, B * NST], F32)
            sall = pp.tile([J, B * NST], F32)
            cscr = pp.tile([J, B * J], F32)
            ct = pp.tile([J, B * J], F32)
            out_sb = pp.tile([J, B * (D2 + 1)], F32)
            # per-iteration small tensors (overwritten each iter)
            s0e = pp.tile([J, B], F32)
            s0r = pp.tile([J, B], F32)
            mu = pp.tile([J, B * D2], F32)
            t_a = pp.tile([J, B * D2], F32)
            t_b = pp.tile([J, B * D2], F32)
            sig = pp.tile([J, B * D2], F32)
            lsig = pp.tile([J, B * D2], F32)
            isig = pp.tile([J, B * D2], F32)
            m1 = pp.tile([J, B * D2], F32)
            m1t = pp.tile([J, B * D2], F32)
            isigt = pp.tile([J, B * D2], F32)
            m1th = pp.tile([J, B * D2], F16)
            isigth = pp.tile([J, B * D2], F16)
            ls = pp.tile([J, B], F32)
            costf = pp.tile([J, B], F32)
            costt = pp.tile([J, B], F32)
            a_j = pp.tile([J, B], F32)
            la = pp.tile([J, B], F32)
            cj = pp.tile([J, B], F32)
            ba_l = pp.tile([J, 1], F32)
            eps1 = pp.tile([J, 1], F32)
            csh1 = pp.tile([J, 1], F32)
            tmp_u = pp.tile([D2, B * cpc * J * D], F32)
            tmp_ur = pp.tile([D2, B * cpc * J], F32)
            tmp_2 = pp.tile([4 * NPAIR, B * cpc * J * D], F32)
            tmp_2r = pp.tile([4 * NPAIR, B * cpc * J], F32)
            tt_c = pp.tile([J, B * D * cpc * D], F32)       # contraction tmp u
            tt_h = pp.tile([J, B * D * cpc * NPAIR], F32)   # contraction tmp u2

            # PSUM persistents: per-b stats tiles (+ reused for M0 row 0)
            stats_ps = [ps_st.tile([J, cpc * VF], F32, name=f"stps{bb}")
                        for bb in range(B)]
            qbf1_ps = ps_misc.tile([D2, B * D * J], F32, name="qbf1")
            qbf2_ps = ps_misc.tile([4 * NPAIR, B * D * J], F32, name="qbf2")

            # ---------- load ----------
            usv = u_stage.rearrange("p (b c td f) -> p b c td f",
                                    b=B, c=cpc, td=tdiv)
            psv = pose_s.rearrange("b (td tm) c f -> tm b c td f", tm=P)
            for bb in range(B):
                for cc in range(cpc):
                    nc.sync.dma_start(out=usv[:, bb, cc], in_=psv[:, bb, cc])
            acv = acts.rearrange("p (b c td) -> p b c td", b=B, c=cpc)
            asv = act_s.rearrange("b (td tm) c -> tm b c td", tm=P)
            for bb in range(B):
                nc.sync.dma_start(out=acv[:, bb], in_=asv[:, bb])
            for dst, src in [(sb_wjm, w_jm), (sb_wwjm, ww_jm), (sb_wf, w_f),
                             (sb_ww2f, ww2_f), (sb_bv, beta_v_j),
                             (sb_ba, beta_a_j), (sb_id, id128h),
                             (sb_qsel, qsel), (sb_sel40, sel40),
                             (sb_ones32, ones32)]:
                nc.sync.dma_start(out=dst[:, :], in_=src[:, :])

            Vv = V.rearrange("p (n f) -> p n f", f=VF)
            # zero pads, ones col, u cols (fp16), u2 products
            nc.vector.memset(Vv[:, :, 40:64], 0.0)
            nc.vector.memset(Vv[:, :, 80:96], 0.0)
            nc.vector.memset(Vv[:, :, F_ONE:F_ONE + 1], 1.0)
            Vv5 = V.rearrange("p (b c td f) -> p b c td f",
                              b=B, c=cpc, td=tdiv)
            for bb in range(B):
                for cc in range(cpc):
                    nc.vector.tensor_copy(
                        Vv5[:, bb, cc, :, F_U:F_U + D2],
                        usv[:, bb, cc])
            for k, (q, qp) in enumerate(PAIRS):
                nc.vector.tensor_mul(
                    Vv[:, :, 4 * k: 4 * k + 4],
                    Vv[:, :, F_U + 4 * q: F_U + 4 * q + 4],
                    Vv[:, :, F_U + 4 * qp: F_U + 4 * qp + 4])
            aJv = actsJ.rearrange("p (b c td) -> p b c td", b=B, c=cpc)
            for bb in range(B):
                nc.vector.tensor_scalar_mul(aJv[:, bb], acv[:, bb], 1.0 / J)
            nc.vector.tensor_reduce(bvsum[:, :], sb_bv[:, :], axis=AX.X,
                                    op=ALU.add)
            nc.vector.memset(cscr[:, :], 0.0)
            nc.vector.memset(eps1[:, :], EPS)
            nc.vector.memset(csh1[:, :], -C_SHIFT)

            # ---------- helpers ----------
            def contraction(src_sb):
                """stats [J,(b,c,VF)] j-major -> sred [J,(b,33)] = S0|S1|S2m"""
                sv = src_sb.rearrange("p (b c f) -> p b c f", b=B, c=cpc)
                rv = sred.rearrange("p (b f) -> p b f", b=B)
                w4 = sb_wjm.rearrange("p (pp c q) -> p pp c q", pp=D, c=cpc)
                ww4 = sb_wwjm.rearrange("p (pp c k) -> p pp c k", pp=D, c=cpc)
                tcv = tt_c.rearrange("p (b r c q) -> p b r c q",
                                     b=B, r=D, c=cpc)
                thv = tt_h.rearrange("p (b r c k) -> p b r c k",
                                     b=B, r=D, c=cpc)
                nc.vector.tensor_reduce(rv[:, :, 0:1],
                                        sv[:, :, :, F_ONE:F_ONE + 1],
                                        axis=AX.XY, op=ALU.add)
                for bb in range(B):
                    for pidx in range(D):
                        # S1[b,j,p,r] = sum_{c,q} w[c,j,p,q] G[b,c,j,(q r)]
                        in0 = sv[:, bb, :, F_U:F_U + D2] \
                            .rearrange("p c (q r) -> p r c q", q=D)
                        in1 = w4[:, pidx].unsqueeze(1) \
                            .broadcast_to((J, D, cpc, D))
                        t0 = tcv[:, bb]
                        nc.vector.tensor_tensor(t0, in0, in1, op=ALU.mult)
                        nc.vector.tensor_reduce(
                            rv[:, bb, 1 + 4 * pidx:1 + 4 * pidx + 4], t0,
                            axis=AX.XY, op=ALU.add)
                        # S2m[b,j,p,r] = sum_{c,k} ww[c,j,p,k] H[b,c,j,(k r)]
                        in0h = sv[:, bb, :, F_U2:F_U2 + 4 * NPAIR] \
                            .rearrange("p c (k r) -> p r c k", k=NPAIR)
                        in1h = ww4[:, pidx].unsqueeze(1) \
                            .broadcast_to((J, D, cpc, NPAIR))
                        t1 = thv[:, bb]
                        nc.vector.tensor_tensor(t1, in0h, in1h, op=ALU.mult)
                        nc.vector.tensor_reduce(
                            rv[:, bb, 17 + 4 * pidx:17 + 4 * pidx + 4], t1,
                            axis=AX.XY, op=ALU.add)

            def allreduce(it):
                din = dramp.tile([J, B * NST], F32, name=f"din{it}")
                dout = dramp.tile([J, B * NST], F32, name=f"dout{it}")
                nc.sync.dma_start(out=din[:, :], in_=sred[:, :])
                if collective:
                    nc.gpsimd.collective_compute(
                        "AllReduce", ALU.add,
                        replica_groups=[list(range(n_cores))],
                        ins=[din.opt()], outs=[dout.opt()])
                    nc.sync.dma_start(out=sall[:, :], in_=dout[:, :])
                else:
                    nc.sync.dma_start(out=sall[:, :], in_=din[:, :])

            def post_stats(t, build_wall):
                sv = sall.rearrange("p (b f) -> p b f", b=B)
                s0v = sv[:, :, 0]
                s1v = sv[:, :, 1:17]
                s2v = sv[:, :, 17:33]
                v3 = lambda x: x.rearrange("p (b f) -> p b f", b=B)
                b16 = lambda x: x.unsqueeze(2).broadcast_to((J, B, D2))
                nc.vector.tensor_scalar_add(s0e[:, :], s0v, EPS)
                nc.vector.reciprocal(s0r[:, :], s0e[:, :])
                nc.vector.tensor_tensor(v3(mu), s1v, b16(s0r), op=ALU.mult)
                # sigma2 = (S2m - mu*(2*S1 - mu*S0)) / S0e + EPS
                nc.vector.tensor_tensor(v3(t_a), v3(mu), b16(s0v),
                                        op=ALU.mult)
                nc.vector.tensor_scalar_mul(v3(t_b), s1v, 2.0)
                nc.vector.tensor_sub(t_b[:, :], t_b[:, :], t_a[:, :])
                nc.vector.tensor_tensor(t_a[:, :], mu[:, :], t_b[:, :],
                                        op=ALU.mult)
                nc.vector.tensor_sub(v3(t_a), s2v, v3(t_a))
                nc.vector.tensor_tensor(v3(sig), v3(t_a), b16(s0r),
                                        op=ALU.mult)
                nc.vector.tensor_scalar_add(sig[:, :], sig[:, :], EPS)
                nc.scalar.activation(lsig[:, :], sig[:, :], AF.Ln)
                nc.vector.reciprocal(isig[:, :], sig[:, :])
                nc.vector.tensor_reduce(
                    ls[:, :], lsig.rearrange("p (b f) -> p b f", b=B),
                    axis=AX.X, op=ALU.add)
                # cost_total = S0 * (0.5*sum(log sig) + sum(beta_v))
                nc.scalar.activation(costf[:, :], ls[:, :], AF.Identity,
                                     bias=bvsum[:, 0:1], scale=0.5)
                nc.vector.tensor_tensor(costt[:, :], costf[:, :], s0v,
                                        op=ALU.mult)
                nc.scalar.mul(ba_l[:, :], sb_ba[:, :], float(lam[t]))
                nc.scalar.activation(a_j[:, :], costt[:, :], AF.Sigmoid,
                                     bias=ba_l[:, 0:1], scale=-float(lam[t]))
                if not build_wall:
                    return None
                wall = wallp.tile([VF, B * cpc * J], F16, name="wall")
                nc.vector.memset(wall[:, :], 0.0)
                nc.vector.tensor_mul(m1[:, :], mu[:, :], isig[:, :])
                nc.scalar.activation(la[:, :], a_j[:, :], AF.Ln,
                                     bias=eps1[:, 0:1])
                # C = log(a+eps) - 0.5*sum(mu*m1 + lsig) - C_SHIFT
                nc.vector.tensor_mul(t_a[:, :], mu[:, :], m1[:, :])
                nc.vector.tensor_add(t_a[:, :], t_a[:, :], lsig[:, :])
                nc.vector.tensor_reduce(
                    cj[:, :], t_a.rearrange("p (b f) -> p b f", b=B),
                    axis=AX.X, op=ALU.add)
                nc.scalar.activation(cj[:, :], cj[:, :], AF.Identity,
                                     bias=csh1[:, 0:1], scale=-0.5)
                nc.vector.tensor_add(cj[:, :], cj[:, :], la[:, :])
                # C[b] to row 0 of per-b 32-blocks via stream transpose
                cv = cscr.rearrange("p (b j) -> p b j", b=B)
                for bb in range(B):
                    nc.vector.tensor_copy(cv[:, bb, 0:1],
                                          cj[:, bb:bb + 1])
                nc.vector.transpose(ct[:, :], cscr[:, :])
                wv = wall.rearrange("p (b c j) -> p b c j", b=B, c=cpc)
                ctv = ct.rearrange("p (b j) -> p b j", b=B)
                for bb in range(B):
                    nc.vector.tensor_copy(
                        wv[F_ONE:F_ONE + 1, bb],
                        ctv[0:1, bb].unsqueeze(1)
                        .broadcast_to((1, cpc, J)))
                # M1 / ISIG transposed [32 (b,p,r), 32 j]
                nc.vector.transpose(m1t[:, :], m1[:, :])
                nc.vector.transpose(isigt[:, :], isig[:, :])
                nc.vector.tensor_copy(m1th[:, :], m1t[:, :])
                nc.vector.tensor_copy(isigth[:, :], isigt[:, :])
                # broadcast rows via selector matmuls
                q1 = qbf1_ps.rearrange("p (b pp j) -> p b pp j", b=B, pp=D)
                q2 = qbf2_ps.rearrange("p (b pp j) -> p b pp j", b=B, pp=D)
                for bb in range(B):
                    for pidx in range(D):
                        i0 = (bb * D + pidx) * D2
                        nc.tensor.matmul(
                            q1[:, bb, pidx],
                            sb_qsel[:, i0:i0 + D2],
                            m1th[:, 0:J], start=True, stop=True)
                        i1 = (bb * D + pidx) * 4 * NPAIR
                        nc.tensor.matmul(
                            q2[:, bb, pidx],
                            sb_sel40[:, i1:i1 + 4 * NPAIR],
                            isigth[:, 0:J], start=True, stop=True)
                # WALL u-part: rows 64..80 = sum_p w_f * M1_qbf
                w_f_ap = sb_wf.rearrange("p (pp c j) -> p pp c j",
                                         pp=D, c=cpc).transpose([0, 2, 3, 1])
                ww2_ap = sb_ww2f.rearrange("p (pp c j) -> p pp c j",
                                           pp=D, c=cpc).transpose([0, 2, 3, 1])
                tuv = tmp_u.rearrange("p (b c j pp) -> p b c j pp",
                                      b=B, c=cpc, j=J)
                turv = tmp_ur.rearrange("p (b c j) -> p b c j", b=B, c=cpc)
                t2v = tmp_2.rearrange("p (b c j pp) -> p b c j pp",
                                      b=B, c=cpc, j=J)
                t2rv = tmp_2r.rearrange("p (b c j) -> p b c j", b=B, c=cpc)
                for bb in range(B):
                    m1q = q1[:, bb].transpose([0, 2, 1]).unsqueeze(1) \
                        .broadcast_to((D2, cpc, J, D))
                    nc.vector.tensor_tensor(tuv[:, bb], w_f_ap, m1q,
                                            op=ALU.mult)
                    nc.vector.tensor_reduce(turv[:, bb], tuv[:, bb],
                                            axis=AX.X, op=ALU.add)
                    # WALL u2-part: rows 0..40 = sum_p ww2_f * ISIG_pair
                    isq = q2[:, bb].transpose([0, 2, 1]).unsqueeze(1) \
                        .broadcast_to((4 * NPAIR, cpc, J, D))
                    nc.vector.tensor_tensor(t2v[:, bb], ww2_ap, isq,
                                            op=ALU.mult)
                    nc.vector.tensor_reduce(t2rv[:, bb], t2v[:, bb],
                                            axis=AX.X, op=ALU.add)
                nc.vector.tensor_copy(wall[F_U:F_U + D2, :], tmp_ur[:, :])
                nc.vector.tensor_copy(wall[F_U2:F_U2 + 4 * NPAIR, :],
                                      tmp_2r[:, :])
                return wall

            # ---------- phase 0: M-step with uniform r ----------
            for bb in range(B):
                for cc in range(cpc):
                    for td in range(tdiv):
                        i = (bb * cpc + cc) * tdiv + td
                        nc.tensor.matmul(
                            stats_ps[bb][0:1, cc * VF:(cc + 1) * VF],
                            actsJ[:, i:i + 1], V[:, i * VF:(i + 1) * VF],
                            start=(td == 0), stop=(td == tdiv - 1))
            m0v = m0_sb.rearrange("p (b f) -> p b f", b=B)
            for bb in range(B):
                nc.vector.tensor_copy(m0v[:, bb], stats_ps[bb][0:1, :])
            for bb in range(B):
                nc.tensor.matmul(stats_ps[bb][:, :], sb_ones32[:, :],
                                 m0v[:, bb], start=True, stop=True)
            ssv = stats_sb.rearrange("p (b x) -> p b x", b=B)
            for bb in range(B):
                nc.vector.tensor_copy(ssv[:, bb], stats_ps[bb][:, :])

            # ---------- f-major transposes (overlap with AR0/post0) ----------
            for i in range(ntile):
                tr = ps_tr.tile([VF, P], F16, name="tr")
                nc.tensor.transpose(tr[:, :], V[:, i * VF:(i + 1) * VF],
                                    sb_id[:, :])
                nc.vector.tensor_copy(U2U[:, i * P:(i + 1) * P], tr[:, :])

            contraction(stats_sb)
            allreduce(0)
            wall = post_stats(0, build_wall=True)

            # ---------- EM passes ----------
            av = acts.rearrange("p (b c td) -> p b c td", b=B, c=cpc)
            for t in range(1, N_ITER):
                for bb in range(B):
                    for cc in range(cpc):
                        bc = bb * cpc + cc
                        for g in range(ngrp):
                            lg = ps_lg.tile([P, GW * J], F32, name="lg")
                            for k in range(GW):
                                i = bc * tdiv + g * GW + k
                                nc.tensor.matmul(
                                    lg[:, k * J:(k + 1) * J],
                                    U2U[:, i * P:(i + 1) * P],
                                    wall[:, bc * J:(bc + 1) * J],
                                    start=True, stop=True)
                            e = wp.tile([P, GW * J], F32, name="e")
                            nc.scalar.activation(e[:, :], lg[:, :], AF.Exp)
                            ev = e.rearrange("p (g j) -> p g j", g=GW)
                            rs = wp.tile([P, GW], F32, name="rs")
                            nc.vector.tensor_reduce(rs[:, :], ev, axis=AX.X,
                                                    op=ALU.add)
                            rsi = wp.tile([P, GW], F32, name="rsi")
                            nc.vector.reciprocal(rsi[:, :], rs[:, :])
                            sc = wp.tile([P, GW], F32, name="sc")
                            nc.vector.tensor_mul(
                                sc[:, :], rsi[:, :],
                                av[:, bb, cc, g * GW:g * GW + GW])
                            ra = wp.tile([P, GW * J], F16, name="ra")
                            nc.vector.tensor_tensor(
                                ra.rearrange("p (g j) -> p g j", g=GW), ev,
                                sc.unsqueeze(2).broadcast_to((P, GW, J)),
                                op=ALU.mult)
                            for k in range(GW):
                                td = g * GW + k
                                i = bc * tdiv + td
                                nc.tensor.matmul(
                                    stats_ps[bb][:, cc * VF:(cc + 1) * VF],
                                    ra[:, k * J:(k + 1) * J],
                                    V[:, i * VF:(i + 1) * VF],
                                    start=(td == 0), stop=(td == tdiv - 1))
                for bb in range(B):
                    nc.vector.tensor_copy(ssv[:, bb], stats_ps[bb][:, :])
                contraction(stats_sb)
                allreduce(t)
                wall = post_stats(t, build_wall=(t < N_ITER - 1))

            # ---------- output ----------
            ov = out_sb.rearrange("p (b f) -> p b f", b=B)
            nc.vector.tensor_copy(
                ov[:, :, 0:D2], mu.rearrange("p (b f) -> p b f", b=B))
            nc.vector.tensor_copy(ov[:, :, D2:D2 + 1],
                                  a_j.unsqueeze(2))
            nc.sync.dma_start(out=caps.transpose([1, 0, 2]), in_=ov)

    nc.finalize()
    return nc


def _host_statics(w, beta_v, beta_a, core, cpc=CPC):
    """Per-core static tensors derived from learned params."""
    wc = np.ascontiguousarray(w[core * cpc:(core + 1) * cpc], np.float32)
    # w_jm [J, (p, c, q)]
    w_jm = np.ascontiguousarray(wc.transpose(1, 2, 0, 3)).reshape(J, -1)
    # ww_jm [J, (p, c, k)] with (2-delta) folded
    ww = np.stack([wc[:, :, :, q] * wc[:, :, :, qp] for (q, qp) in PAIRS],
                  axis=-1) * PAIR_W  # [c, J, p, k]
    ww_jm = np.ascontiguousarray(ww.transpose(1, 2, 0, 3)).reshape(J, -1)
    # w_f [(q,r), (p, c, j)] (r-broadcast of w[c,j,p,q])
    w_f = np.empty((D2, D, cpc, J), np.float32)
    for q in range(D):
        for r in range(D):
            # [p, c, j] = wc[c, j, p, q] -> transpose
            w_f[q * 4 + r] = wc[:, :, :, q].transpose(2, 0, 1)
    w_f = w_f.reshape(D2, -1)
    # ww2_f [(k,r) 40, (p, c, j)] values -0.5*(2-delta)*w_q*w_qp
    ww2 = np.empty((4 * NPAIR, D, cpc, J), np.float32)
    for k, (q, qp) in enumerate(PAIRS):
        val = (-0.5 * PAIR_W[k]) * wc[:, :, :, q] * wc[:, :, :, qp]  # [c,J,p]
        for r in range(D):
            ww2[4 * k + r] = val.transpose(2, 0, 1)
    ww2_f = ww2.reshape(4 * NPAIR, -1)
    qsel = np.zeros((J, B * D * D2), np.float32)
    sel40 = np.zeros((J, B * D * 4 * NPAIR), np.float32)
    for b in range(B):
        for p in range(D):
            for r in range(D):
                row = 16 * b + 4 * p + r
                for q in range(D):
                    qsel[row, (b * D + p) * D2 + q * 4 + r] = 1.0
                for k in range(NPAIR):
                    sel40[row, (b * D + p) * 4 * NPAIR + 4 * k + r] = 1.0
    return {
        "w_jm": w_jm, "ww_jm": ww_jm, "w_f": w_f, "ww2_f": ww2_f,
        "beta_v_j": np.ascontiguousarray(beta_v, np.float32).reshape(J, D2),
        "beta_a_j": np.ascontiguousarray(beta_a, np.float32).reshape(J, 1),
        "id128h": np.eye(P, dtype=np.float16),
        "qsel": qsel.astype(np.float16), "sel40": sel40.astype(np.float16),
        "ones32": np.ones((1, J), np.float16),
    }


_CACHE = {}


def kernel(pose, activation, w, beta_v, beta_a):
    pose = np.ascontiguousarray(np.asarray(pose), dtype=np.float32)
    activation = np.ascontiguousarray(np.asarray(activation), np.float32)
    w = np.asarray(w, dtype=np.float32)
    beta_v = np.asarray(beta_v, np.float32)
    beta_a = np.asarray(beta_a, np.float32)

    if "nc" not in _CACHE:
        _CACHE["nc"] = _build_nc()
    nc = _CACHE["nc"]

    in_maps = []
    for core in range(N_CORES):
        sl = slice(core * CPC, (core + 1) * CPC)
        m = {
            "pose_s": np.ascontiguousarray(pose[:, :, sl, :]),
            "act_s": np.ascontiguousarray(activation[:, :, sl, 0]),
        }
        m.update(_host_statics(w, beta_v, beta_a, core))
        in_maps.append(m)

    trace = bool(int(os.environ.get("KERNEL_TRACE", "0")))
    try:
        res = run_bass_kernel_spmd(nc, in_maps,
                                   core_ids=list(range(N_CORES)),
                                   trace=trace)
    except ModuleNotFoundError:
        res = run_bass_kernel_spmd(nc, in_maps,
                                   core_ids=list(range(N_CORES)),
                                   trace=False)
    if trace and res.exec_time_ns is not None:
        print(f"HW exec time: {res.exec_time_ns} ns")
        _CACHE["exec_time_ns"] = res.exec_time_ns
        _CACHE["results"] = res
    caps = np.asarray(res.results[0]["caps"], np.float32)
    return (caps, caps.copy())


if __name__ == "__main__":
    rng = np.random.default_rng(0)
    inputs = {
        "pose": rng.standard_normal((B, NI, CH, D2), dtype=np.float32),
        "activation": rng.random((B, NI, CH, 1), dtype=np.float32),
        "w": 0.1 * rng.standard_normal((CH, J, D, D), dtype=np.float32),
        "beta_v": 0.1 * rng.standard_normal((J, D2), dtype=np.float32),
        "beta_a": 0.1 * rng.standard_normal((J, 1), dtype=np.float32),
    }
    out = kernel(**inputs)
    print(out[0].shape, out[0][0, 0])
